# revision 22
# baseline (speedup 1.0000x reference)
"""Trainium2 Bass kernel for the ChebConv GNN problem
(nn_ChebConvConvolutional): 2x GCNConv + 1x ChebConv(K=3), N=10000 nodes,
E=160000 edges, F=512, celu activations.

Strategy (8 NeuronCores, SPMD):
  * Nodes are sharded 1250/core (padded to 1280). Edges are sharded by
    destination core and grouped into 128-dest tiles; per dest-tile the
    source nodes are deduplicated and the edge weights are baked into dense
    [128 src x 128 dst] one-hot "S" matrices (GCN self-loops folded in as
    edges with value dinv^2, Cheb normalization negated so the scatter
    directly produces lhat).
  * Every graph op is computed aggregate-first: h = celu((A @ x) @ W + b),
    so layer 1 needs no collective (x replicated); layers end with a small
    AllGather of the core's 1280-row bf16 slice.
  * On device, per dest-tile: dma_gather pulls the (bf16) feature rows of
    the deduped sources; the tensor engine computes
    ps[d, f] += S[e, d].T @ msgs[e, f], then dense GEMMs, and
    celu = max(z,0) + min(exp(z)-1, 0) runs on ACT + DVE.
  * ChebConv K=3 uses the commuted form (lhat is row-mixing, W col-mixing):
        out = celu(h2 @ Wa + lhat(h2 @ Wb) + lhat(lhat(h2 @ Wc2)) + bc)
    with Wa = Wk0 - Wk2, Wb = Wk1, Wc2 = 2*Wk2. After layer 2 each core
    projects its h2 tiles to pq = h2 @ [Wb | Wc2] (512 wide), AllGathers pq
    once; pass A scatters pq into [lhat_p | lhat_q], keeps lhat_p on chip
    and AllGathers r = lhat_q (256 wide); pass B scatters r (half-width
    gather) and finishes out = celu(psum + h2 @ Wa + lhat_p + bc) in psum.
"""
import numpy as np
import ml_dtypes

import concourse.bacc as bacc
import concourse.mybir as mybir
import concourse.tile as tile
from concourse import library_config
from concourse.bass_utils import run_bass_kernel_spmd
from concourse.tile import add_dep_helper

BF16 = ml_dtypes.bfloat16
FP32 = mybir.dt.float32
BF16D = mybir.dt.bfloat16
I16 = mybir.dt.int16

P = 8            # cores
N = 10000        # nodes
NPC = N // P     # nodes per core
NPAD = 1280      # padded nodes per core
NTOT = NPAD * P
F = 512          # feature width of x / h1 / h2
DOUT = 256
DT = 128         # dests per dest tile
NDT = NPAD // DT # dest tiles per core
KC = F // 128    # contraction chunks (4)


# ----------------------------------------------------------------- host prep

def _to_padded_id(n):
    """Global node id -> row in the AllGather global layout: [P ranks][NPAD]."""
    r = n // NPC
    return r * NPAD + (n % NPC)


def _build_edge_tiles(src, dst, val):
    """Shard by dest core, tile by 128 dests, dedup sources per tile.
    Returns (ET [NDT], idx [P, T, 128] int32 padded ids, S [P, T, 128, DT])."""
    per_core = []
    order = np.argsort(dst, kind="stable")
    src, dst, val = src[order], dst[order], val[order]
    core_of = dst // NPC
    core_starts = np.searchsorted(core_of, np.arange(P + 1))
    for c in range(P):
        lo, hi = core_starts[c], core_starts[c + 1]
        s, d, v = src[lo:hi], dst[lo:hi] - c * NPC, val[lo:hi]
        tile_of = d // DT
        tile_starts = np.searchsorted(tile_of, np.arange(NDT + 1))
        groups = []
        for t in range(NDT):
            a, b = tile_starts[t], tile_starts[t + 1]
            st, dl, vt = s[a:b], d[a:b] - t * DT, v[a:b]
            uniq, inv = np.unique(st, return_inverse=True)
            if len(uniq) == 0:
                groups.append((np.zeros(1, np.int64), np.zeros((1, DT), np.float32)))
                continue
            S = np.zeros((len(uniq), DT), np.float32)
            np.add.at(S, (inv, dl), vt)
            groups.append((uniq, S))
        per_core.append(groups)

    ET = [max(max((len(per_core[c][t][0]) + 127) // 128, 1) for c in range(P))
          for t in range(NDT)]
    T = sum(ET)
    off = np.cumsum([0] + ET[:-1])
    idx = np.zeros((P, T, 128), np.int32)
    S_all = np.zeros((P, T, 128, DT), np.float32)
    for c in range(P):
        for t in range(NDT):
            uniq, S = per_core[c][t]
            n = len(uniq)
            o = off[t]
            idx[c, o:o + (n + 127) // 128].reshape(-1)[:n] = _to_padded_id(uniq)
            S_all[c, o:o + (n + 127) // 128].reshape(-1, DT)[:n] = S
    return tuple(ET), idx, S_all


def _idx_dev(idx_core):
    """[T, 128] int32 -> [128, T*8] int16 (wrap 16 partitions, replicate x8)."""
    flat = idx_core.reshape(-1)
    n = len(flat)
    a = np.zeros((16, n // 16), np.int16)
    a[np.arange(n) % 16, np.arange(n) // 16] = flat.astype(np.int16)
    return np.tile(a, (8, 1))


def _s_dev(S_core):
    """[T, 128, DT] -> [128, T*DT] bf16."""
    T = S_core.shape[0]
    return np.ascontiguousarray(
        S_core.transpose(1, 0, 2).reshape(128, T * DT)).astype(BF16)


def _w_dev(W):
    """[F, fo] -> [128, KC*fo] bf16 (chunk k at cols [k*fo, (k+1)*fo))."""
    fi, fo = W.shape
    k = fi // 128
    return np.ascontiguousarray(
        W.reshape(k, 128, fo).transpose(1, 0, 2).reshape(128, k * fo)).astype(BF16)


def _prep(x, edge_index, edge_weight, W1, b1, W2, b2, Wc, bc):
    row = np.asarray(edge_index[0], np.int64)
    col = np.asarray(edge_index[1], np.int64)
    w = np.asarray(edge_weight, np.float32)

    # GCN norm (layers 1 & 2): deg over dest (col) + 1 self loop.
    deg = np.zeros(N, np.float32)
    np.add.at(deg, col, w)
    deg += 1.0
    dinv = (1.0 / np.sqrt(deg)).astype(np.float32)
    g_src = np.concatenate([row, np.arange(N)])
    g_dst = np.concatenate([col, np.arange(N)])
    g_val = np.concatenate([dinv[row] * w * dinv[col], dinv * dinv]).astype(np.float32)

    # Cheb: drop self loops, deg over src (row), negate (lhat = -A_norm).
    keep = row != col
    r0, c0, w0 = row[keep], col[keep], w[keep]
    deg2 = np.zeros(N, np.float32)
    np.add.at(deg2, r0, w0)
    dinv2 = np.where(deg2 > 0, 1.0 / np.sqrt(deg2), 0.0).astype(np.float32)
    c_val = -(dinv2[r0] * w0 * dinv2[c0]).astype(np.float32)

    ETg, idxg, Sg = _build_edge_tiles(g_src, g_dst, g_val)
    ETc, idxc, Sc = _build_edge_tiles(r0, c0, c_val)

    x = np.asarray(x, np.float32)
    x_pad = np.zeros((NTOT, F), BF16)
    x_pad[_to_padded_id(np.arange(N))] = x.astype(BF16)

    Wc = np.asarray(Wc, np.float32)
    com = dict(
        w1=_w_dev(np.asarray(W1, np.float32)),
        w2=_w_dev(np.asarray(W2, np.float32)),
        wa=_w_dev(Wc[0] - Wc[2]),
        wb=_w_dev(Wc[1]),
        wc2=_w_dev(2.0 * Wc[2]),
        ident=np.eye(128, dtype=BF16),
    )
    biases = (np.asarray(b1, np.float32), np.asarray(b2, np.float32),
              np.asarray(bc, np.float32))
    in_maps = []
    for c in range(P):
        m = dict(com)
        # Layer 1 gathers from x (host-known): pre-gather the message rows
        # so the device streams them with static DMA (no descriptor gen).
        m["msgs1"] = np.ascontiguousarray(x_pad[idxg[c].reshape(-1)])
        m["idxg"] = _idx_dev(idxg[c])
        m["sg"] = _s_dev(Sg[c])
        m["idxc"] = _idx_dev(idxc[c])
        m["sc"] = _s_dev(Sc[c])
        in_maps.append(m)
    return ETg, ETc, biases, in_maps


# ------------------------------------------------------------- bass program

_CACHE = {}


def _build_program(ETg, ETc, has_bias):
    import os
    key = (ETg, ETc, has_bias, os.environ.get("GNN_PHASES", "9"))
    if key in _CACHE:
        return _CACHE[key]
    TG, TC = sum(ETg), sum(ETc)
    ETMAX = max(max(ETg), max(ETc))
    ETCMAX = max(ETc)

    nc = bacc.Bacc("TRN2", target_bir_lowering=False, num_devices=P,
                   num_swdge_queues=4)
    msgs1 = nc.dram_tensor("msgs1", [TG * 128, F], BF16D, kind="ExternalInput")
    idxg = nc.dram_tensor("idxg", [128, TG * 8], I16, kind="ExternalInput")
    sg = nc.dram_tensor("sg", [128, TG * DT], BF16D, kind="ExternalInput")
    idxc = nc.dram_tensor("idxc", [128, TC * 8], I16, kind="ExternalInput")
    sc = nc.dram_tensor("sc", [128, TC * DT], BF16D, kind="ExternalInput")
    w1 = nc.dram_tensor("w1", [128, KC * F], BF16D, kind="ExternalInput")
    w2 = nc.dram_tensor("w2", [128, KC * F], BF16D, kind="ExternalInput")
    wa = nc.dram_tensor("wa", [128, KC * DOUT], BF16D, kind="ExternalInput")
    wb = nc.dram_tensor("wb", [128, KC * DOUT], BF16D, kind="ExternalInput")
    wc2 = nc.dram_tensor("wc2", [128, KC * DOUT], BF16D, kind="ExternalInput")
    ident = nc.dram_tensor("ident", [128, 128], BF16D, kind="ExternalInput")
    if has_bias:
        brows = nc.dram_tensor("brows", [1, 2 * F + DOUT], FP32, kind="ExternalInput")
    outp = nc.dram_tensor("out", [NPAD, DOUT], FP32, kind="ExternalOutput")

    h1c = nc.dram_tensor("h1c", [NPAD, F], BF16D, kind="Internal")
    h1f = nc.dram_tensor("h1f", [NTOT, F], BF16D, kind="Internal", addr_space="Shared")
    pqc = nc.dram_tensor("pqc", [NPAD, F], BF16D, kind="Internal")
    pqf = nc.dram_tensor("pqf", [NTOT, F], BF16D, kind="Internal", addr_space="Shared")
    rc = nc.dram_tensor("rc", [NPAD, DOUT], BF16D, kind="Internal")
    rf = nc.dram_tensor("rf", [NTOT, DOUT], BF16D, kind="Internal",
                        addr_space="Shared")

    Exp = mybir.ActivationFunctionType.Exp
    Alu = mybir.AluOpType

    with tile.TileContext(nc) as tc:
        with (
            tc.tile_pool(name="const", bufs=1) as cpool,
            tc.tile_pool(name="keep", bufs=1) as kpool,
            tc.tile_pool(name="msgs", bufs=2) as mpool,
            tc.tile_pool(name="work", bufs=3) as wpool,
            tc.tile_pool(name="psum", bufs=2, space="PSUM") as ppool,
            tc.tile_pool(name="psum3", bufs=3, space="PSUM") as ppool3,
        ):
            lib = nc.gpsimd.load_library(library_config.mlp)

            id_sb = cpool.tile([128, 128], BF16D, tag="id")
            nc.sync.dma_start(id_sb[:], ident[:])

            ig_sb = cpool.tile([128, TG * 8], I16, tag="ig")
            ic_sb = cpool.tile([128, TC * 8], I16, tag="ic")
            sg_sb = cpool.tile([128, TG * DT], BF16D, tag="sg")
            sc_sb = cpool.tile([128, TC * DT], BF16D, tag="sc")
            offg = np.cumsum([0] + list(ETg[:-1]))
            offc = np.cumsum([0] + list(ETc[:-1]))
            # L1 needs only w1 + the GCN S tiles; everything else loads later
            # (after the L1 streams are underway) so L1 starts immediately.
            w1_sb = cpool.tile([128, KC * F], BF16D, tag="w1")
            nc.sync.dma_start(w1_sb[:], w1[:])
            for t in range(NDT):
                a, b = offg[t] * DT, (offg[t] + ETg[t]) * DT
                nc.sync.dma_start(sg_sb[:, a:b], sg[:, a:b])
            if has_bias:
                br_sb = cpool.tile([1, 2 * F + DOUT], FP32, tag="br")
                nc.sync.dma_start(br_sb[:], brows[:])
                ones_sb = cpool.tile([1, 128], FP32, tag="ones")
                nc.vector.memset(ones_sb[:], 1.0)

            w2_sb = cpool.tile([128, KC * F], BF16D, tag="w2")
            wa_sb = cpool.tile([128, KC * DOUT], BF16D, tag="wa")
            wb_sb = cpool.tile([128, KC * DOUT], BF16D, tag="wb")
            wc2_sb = cpool.tile([128, KC * DOUT], BF16D, tag="wc2")

            def late_loads():
                """Small load steps, drained a couple per L1 tile so they
                interleave with (rather than block) the msgs1 streams."""
                yield lambda: nc.sync.dma_start(w2_sb[:], w2[:])
                yield lambda: nc.sync.dma_start(ig_sb[:], idxg[:])
                yield lambda: nc.sync.dma_start(wa_sb[:], wa[:])
                yield lambda: nc.sync.dma_start(wb_sb[:], wb[:])
                yield lambda: nc.sync.dma_start(wc2_sb[:], wc2[:])
                yield lambda: nc.sync.dma_start(ic_sb[:], idxc[:])
                for t in range(NDT):
                    a, b = offc[t] * DT, (offc[t] + ETc[t]) * DT
                    yield lambda a=a, b=b: nc.sync.dma_start(
                        sc_sb[:, a:b], sc[:, a:b])

            h2keep = kpool.tile([128, NDT, F], BF16D, tag="h2k")
            lapkeep = kpool.tile([128, NDT, DOUT], BF16D, tag="lap")

            first_gather = [0]
            qctr = [0]

            def scatter(src_dram, ET, off, idx_sb, s_sb, t, width=F,
                        msgs_tag="msgs", msgs_w=None, stop_at_end=True):
                """Gather + one-hot matmuls for dest-tile t.
                Returns psum view [128, width]: [dest, feature]."""
                o = off[t]
                et = ET[t]
                mw = msgs_w if msgs_w is not None else ETMAX
                msgs = mpool.tile([128, mw, width], BF16D, tag=msgs_tag)
                nq = min(4, et)
                bounds = [et * i // nq for i in range(nq + 1)]
                for a, b in zip(bounds[:-1], bounds[1:]):
                    if b <= a:
                        continue
                    q = qctr[0] % 4
                    qctr[0] += 1
                    gi = nc.gpsimd.dma_gather(
                        msgs[:, a:b, :], src_dram[:],
                        idx_sb[:, (o + a) * 8:(o + b) * 8],
                        (b - a) * 128, (b - a) * 128, width,
                        single_packet=False, queue_num=q)
                    if first_gather[0] < 4:
                        add_dep_helper(gi.ins, lib.ins,
                                       reason="mlp lib before gather")
                        first_gather[0] += 1
                # S tile as stationary lhsT, msgs streamed as rhs:
                # ps[d, f] += S[e, d].T @ msgs[e, f]  (node-major aggregate).
                pst = ppool3.tile([128, F], FP32, tag="psT")
                for g in range(et):
                    nc.tensor.matmul(
                        pst[:, :width],
                        s_sb[:, (o + g) * DT:(o + g + 1) * DT],
                        msgs[:, g, :],
                        start=(g == 0),
                        stop=(stop_at_end and g == et - 1))
                return pst

            def celu(z_ps, width, out_ap):
                """out = max(z,0) + min(exp(z)-1, 0); z read from PSUM."""
                e = wpool.tile([128, F], FP32, tag="e")
                nc.scalar.activation(e[:, :width], z_ps, Exp)
                em = wpool.tile([128, F], FP32, tag="em")
                nc.vector.tensor_scalar(
                    em[:, :width], e[:, :width], 1.0, 0.0,
                    Alu.subtract, Alu.min)
                nc.vector.scalar_tensor_tensor(
                    out_ap, z_ps, 0.0, em[:, :width], Alu.max, Alu.add)

            def gemm_bias(z_ps, width, b_off, stop=False):
                if has_bias:
                    nc.tensor.matmul(
                        z_ps, ones_sb[:],
                        br_sb[:, b_off:b_off + width],
                        start=False, stop=stop)

            def allgather(cin, cout):
                nc.gpsimd.collective_compute(
                    "AllGather", Alu.bypass,
                    replica_groups=[list(range(P))],
                    ins=[cin[:]],
                    outs=[cout[:]])

            import os
            PH = int(os.environ.get("GNN_PHASES", "9"))

            def stream_scatter(t):
                """L1 variant of scatter: stream host-pregathered x rows
                (contiguous, static HWDGE DMA on two queues) + S matmuls."""
                from concourse.ap import AP as _AP
                o = offg[t]
                et = ETg[t]
                msgs = mpool.tile([128, ETMAX, F], BF16D, tag="msgs")
                ha = (et + 1) // 2
                for eng, a, b in ((nc.sync, 0, ha), (nc.scalar, ha, et)):
                    if b <= a:
                        continue
                    eng.dma_start(
                        msgs[:, a:b, :],
                        _AP(msgs1, int(o + a) * 128 * F,
                            [[F, 128], [128 * F, b - a], [1, F]]))
                pst = ppool3.tile([128, F], FP32, tag="psT")
                for g in range(et):
                    nc.tensor.matmul(
                        pst[:],
                        sg_sb[:, (o + g) * DT:(o + g + 1) * DT],
                        msgs[:, g, :],
                        start=(g == 0), stop=(g == et - 1))
                return pst

            # ---- layer 1: h1 = celu((Ag @ x) @ W1 + b1)
            _late = late_loads()
            for t in range(NDT):
                ps = stream_scatter(t)
                agg = wpool.tile([128, F], BF16D, tag="agg")
                nc.vector.tensor_copy(agg[:], ps[:])
                tps = ppool.tile([128, KC, 128], BF16D, tag="tps")
                for k in range(KC):
                    nc.tensor.transpose(
                        tps[:, k, :], agg[:, k * 128:(k + 1) * 128], id_sb[:])
                aggT = wpool.tile([128, KC, 128], BF16D, tag="aggT")
                nc.vector.tensor_copy(aggT[:], tps[:])
                z = ppool.tile([128, F], FP32, tag="z")
                for k in range(KC):
                    nc.tensor.matmul(
                        z[:], aggT[:, k, :], w1_sb[:, k * F:(k + 1) * F],
                        start=(k == 0), stop=False)
                gemm_bias(z[:], F, 0, stop=True)
                h = wpool.tile([128, F], BF16D, tag="h")
                celu(z[:], F, h[:])
                nc.sync.dma_start(h1c[t * 128:(t + 1) * 128, :], h[:])
                if t >= 2:
                    for _ in range(2):
                        step = next(_late, None)
                        if step is not None:
                            step()
            for step in _late:
                step()
            if PH >= 2:
                allgather(h1c, h1f)

            # ---- layer 2: h2 = celu((Ag @ h1) @ W2 + b2), kept on chip.
            #      Per tile, also transpose h2 to feature-major and project
            #      pq = h2 @ [Wb | Wc2] for the cheb passes.
            if PH >= 3:
                for t in range(NDT):
                    ps = scatter(h1f, ETg, offg, ig_sb, sg_sb, t)
                    agg = wpool.tile([128, F], BF16D, tag="agg")
                    nc.vector.tensor_copy(agg[:], ps[:])
                    tps = ppool.tile([128, KC, 128], BF16D, tag="tps")
                    for k in range(KC):
                        nc.tensor.transpose(
                            tps[:, k, :], agg[:, k * 128:(k + 1) * 128], id_sb[:])
                    aggT = wpool.tile([128, KC, 128], BF16D, tag="aggT")
                    nc.vector.tensor_copy(aggT[:], tps[:])
                    z = ppool.tile([128, F], FP32, tag="z")
                    for k in range(KC):
                        nc.tensor.matmul(
                            z[:], aggT[:, k, :], w2_sb[:, k * F:(k + 1) * F],
                            start=(k == 0), stop=False)
                    gemm_bias(z[:], F, F, stop=True)
                    h2t = wpool.tile([128, F], BF16D, tag="h")
                    celu(z[:], F, h2t[:])
                    # feature-major h2 for the three h2 @ W GEMM terms
                    tps2 = ppool.tile([128, KC, 128], BF16D, tag="tps")
                    for k in range(KC):
                        nc.tensor.transpose(
                            tps2[:, k, :], h2t[:, k * 128:(k + 1) * 128],
                            id_sb[:])
                    nc.vector.tensor_copy(h2keep[:, t, :], tps2[:])
                    if PH >= 4:
                        # pq = h2 @ [Wb | Wc2]  (node-major in psum)
                        zpq = ppool.tile([128, F], FP32, tag="z")
                        for k in range(KC):
                            nc.tensor.matmul(
                                zpq[:, 0:DOUT],
                                h2keep[:, t, k * 128:(k + 1) * 128],
                                wb_sb[:, k * DOUT:(k + 1) * DOUT],
                                start=(k == 0), stop=(k == KC - 1))
                        for k in range(KC):
                            nc.tensor.matmul(
                                zpq[:, DOUT:2 * DOUT],
                                h2keep[:, t, k * 128:(k + 1) * 128],
                                wc2_sb[:, k * DOUT:(k + 1) * DOUT],
                                start=(k == 0), stop=(k == KC - 1))
                        pq = wpool.tile([128, F], BF16D, tag="h")
                        nc.vector.tensor_copy(pq[:], zpq[:])
                        nc.sync.dma_start(pqc[t * 128:(t + 1) * 128, :], pq[:])
                if PH >= 4:
                    allgather(pqc, pqf)

            # ---- cheb pass A: [lhat_p | lhat_q] = lhat(pq); keep lhat_p,
            #      AllGather r = lhat_q (256 wide).
            if PH >= 5:
                for t in range(NDT):
                    ps = scatter(pqf, ETc, offc, ic_sb, sc_sb, t)
                    nc.vector.tensor_copy(lapkeep[:, t, :], ps[:, 0:DOUT])
                    rt = wpool.tile([128, DOUT], BF16D, tag="rt")
                    nc.vector.tensor_copy(rt[:], ps[:, DOUT:2 * DOUT])
                    nc.sync.dma_start(rc[t * 128:(t + 1) * 128, :], rt[:])
                allgather(rc, rf)

            # ---- cheb pass B + output:
            # out = celu(lhat(r) + h2 @ Wa + lhat_p + bc)
            if PH >= 6:
                # Local terms zoL = h2 @ Wa + lhat_p + bc precomputed on PE
                # while the r AllGather is in flight.
                zokeep = kpool.tile([128, NDT, DOUT], FP32, tag="zok")
                for t in range(NDT):
                    zl = ppool.tile([128, F], FP32, tag="z")
                    zv = zl[:, :DOUT]
                    for k in range(KC):
                        nc.tensor.matmul(
                            zv, h2keep[:, t, k * 128:(k + 1) * 128],
                            wa_sb[:, k * DOUT:(k + 1) * DOUT],
                            start=(k == 0), stop=False)
                    nc.tensor.matmul(
                        zv, id_sb[:], lapkeep[:, t, :],
                        start=False, stop=not has_bias)
                    gemm_bias(zv, DOUT, 2 * F, stop=True)
                    nc.vector.tensor_copy(zokeep[:, t, :], zv)
                for t in range(NDT):
                    ps = scatter(rf, ETc, offc, ic_sb, sc_sb, t, width=DOUT,
                                 msgs_tag="msgs256", msgs_w=ETCMAX)
                    zs = wpool.tile([128, DOUT], FP32, tag="zs")
                    nc.vector.tensor_add(zs[:], ps[:, :DOUT], zokeep[:, t, :])
                    of = wpool.tile([128, DOUT], FP32, tag="of")
                    celu(zs[:], DOUT, of[:])
                    nc.sync.dma_start(outp[t * 128:(t + 1) * 128, :], of[:])

    nc.compile()
    _CACHE[key] = nc
    return nc


# ------------------------------------------------------------------- driver

def _run(inputs, trace=False, tmpdir=None):
    ETg, ETc, biases, in_maps = _prep(**inputs)
    has_bias = any(np.any(b != 0) for b in biases)
    if has_bias:
        brow = np.concatenate(biases).astype(np.float32)[None, :]
        for m in in_maps:
            m["brows"] = brow
    nc = _build_program(ETg, ETc, has_bias)
    res = run_bass_kernel_spmd(nc, in_maps, core_ids=list(range(P)),
                               trace=trace, tmpdir=tmpdir)
    out = np.concatenate(
        [res.results[c]["out"][:NPC] for c in range(P)], axis=0)
    return out.astype(np.float32), res


def kernel(**inputs) -> np.ndarray:
    out, _ = _run(inputs)
    return out
